# revision 54
# baseline (speedup 1.0000x reference)
"""Trainium2 Bass kernel for causal multi-head attention.

Problem: B=2, T=4096, D=768, H=12 heads, d_k=64, causal mask.
Sharding: 8 cores = 2 batches x 4 head-groups (3 heads each).

v8 design (all-bf16 on device, ~305us vs 400us v2 baseline):
- Host ships x^T (pre-transposed, bf16); weights pre-sliced per head-group.
- One fused loop per 512-query i-chunk: project qk^T/v, flash-style causal
  attention with transposed scores (S^T = k q^T), partial out-projection.
- qk projection ships only 3 channels (ch0=[q1|q2] ch1=[k1|k2] ch2=[q3|k3]);
  the partition-swapped ch3=[k3|q3] needed for pass-B row-packing is built
  by two SBUF partition-shift DMAs instead of 6 extra matmuls per chunk.
- Normalization uses reciprocal_approx_fast (1 DVE op, ~51 ULP) instead of
  the 6-cyc/elem iterative reciprocal that blocked the DVE queue; its src
  is first copied to partition 0 (custom-DVE ops mis-execute on nonzero
  base partitions, HW-verified).
- Deep cross-chunk software pipeline: out-proj of chunk i pumps into pass A
  of chunk i+1, projections of chunk i+1 pump into pass B of chunk i, and
  pass A of chunk i+1 gets a head-start score block inside pass B of chunk
  i, so ScalarE (exp, the ~219us bottleneck engine) stays fed.
- Attention matmuls carry a scheduler priority boost that keeps row-packed
  score pairs adjacent (the Tile priority heap otherwise splits ~half of
  them with ready out-proj matmuls, serializing the concurrent halves).
- Causal handling at 128-block granularity; the true-diagonal block is
  masked by gpsimd affine_select after exp.
- Host sums the 4 head-group partials per batch and adds the folded bias
  constant (v-bias @ W_out + b_out).  k-bias dropped (softmax-invariant).

Self-contained: hardcodes all shapes; only imports the concourse runtime.
"""

import sys

sys.path.insert(0, "/opt/trn_rl_repo")

from contextlib import ExitStack

import numpy as np
import ml_dtypes

import concourse.bass as bass
import concourse.mybir as mybir
import concourse.tile as tile
from concourse import bacc
from concourse.bass_utils import run_bass_kernel_spmd

F32 = mybir.dt.float32
BF16 = mybir.dt.bfloat16
NPBF16 = ml_dtypes.bfloat16

B, T, D = 2, 4096, 768
H, DK = 12, 64
HPC = 3          # heads per core
N_CORES = 8
ICH_W = 512      # i-chunk width (queries per outer step)
JB_W = 128       # j-block width (keys per matmul)
KT = D // 128    # 6 contraction tiles for the projections
USE_RS = False


def build_program(t=T, use_rs=USE_RS):
    """Build the SPMD Bass program for one core (all cores identical)."""
    n_ich = t // ICH_W
    n_tch = t // 128

    nc = bacc.Bacc("TRN2", target_bir_lowering=False, debug=False,
                   num_devices=N_CORES)

    # x^T: [D, t] bf16, row-major (row stride t)
    xt_d = nc.dram_tensor("xt", [D, t], BF16, kind="ExternalInput").ap()
    # qk projection weights, 3 chunks of 128 output channels:
    # ch0=[q1|q2] ch1=[k1|k2] ch2=[q3|k3]; host-padded to 512 columns so
    # the SBUF tile keeps the 4-channel footprint (shrinking it shifts all
    # downstream pools and was measured to slow ACTIVATE 1.0->1.22 cyc/elem)
    # and the DMA stays fully contiguous
    wqk_d = nc.dram_tensor("wqk", [D, 512], BF16, kind="ExternalInput").ap()
    bqk_d = nc.dram_tensor("bqk", [512], F32, kind="ExternalInput").ap()
    wv_d = nc.dram_tensor("wv", [D, HPC * DK], BF16, kind="ExternalInput").ap()
    wout_d = nc.dram_tensor("wout", [HPC * DK, D], BF16,
                            kind="ExternalInput").ap()
    out_d = nc.dram_tensor("out", [t, D], BF16, kind="ExternalOutput").ap()

    with tile.TileContext(nc) as tc, ExitStack() as top:
        consts = top.enter_context(tc.tile_pool(name="consts", bufs=1))
        persist = top.enter_context(tc.tile_pool(name="persist", bufs=1))

        # q^T / k^T per chunk: [128, 4, t] bf16 (ch3 filled by DMA shifts)
        qk_sb = persist.tile([128, 4, t], BF16)
        # v (natural layout) + ones column: [128, n_tch, HPC, 65] bf16
        vaug_sb = persist.tile([128, n_tch, HPC, DK + 1], BF16)

        xtp = top.enter_context(tc.tile_pool(name="xtp", bufs=3))
        xt_tiles = {}

        def emit_xt_dma(ich):
            if ich >= n_ich:
                return
            i0 = ich * ICH_W
            xt = xtp.tile([128, KT, ICH_W], BF16, tag="xt")
            nc.sync.dma_start(
                out=xt,
                in_=xt_d[:, i0:i0 + ICH_W].rearrange(
                    "(kt p) i -> p kt i", p=128))
            xt_tiles[ich] = xt

        # DMA issue order favors the critical path to the first exp:
        # wqk + bqk + xt0 (first qk projections), then everything else
        wqk_sb = consts.tile([128, KT, 512], BF16)
        nc.sync.dma_start(out=wqk_sb,
                          in_=wqk_d.rearrange("(kt p) c -> p kt c", p=128))
        bqk_sb = consts.tile([128, 4], F32)
        nc.sync.dma_start(out=bqk_sb, in_=bqk_d.rearrange("(ch p) -> p ch",
                                                          p=128))
        emit_xt_dma(0)
        wv_sb = consts.tile([128, KT, HPC * DK], BF16)
        nc.sync.dma_start(out=wv_sb,
                          in_=wv_d.rearrange("(kt p) c -> p kt c", p=128))
        emit_xt_dma(1)
        # h0|h1 stacked on 128 partitions (one K=128 out-proj matmul); h2's
        # weights zero-padded to 128 rows so its matmul also runs K=128
        wout01_sb = consts.tile([128, D], BF16)
        nc.sync.dma_start(out=wout01_sb, in_=wout_d[0:128, :])
        wout2_sb = consts.tile([128, D], BF16)
        nc.vector.memset(wout2_sb[64:128, :], 0.0)
        nc.sync.dma_start(out=wout2_sb[0:64, :], in_=wout_d[128:192, :])

        # ones column for the v-augmentation (denominator row), set once
        nc.vector.memset(vaug_sb[:, :, :, DK:DK + 1], 1.0)

        with tc.tile_pool(name="work_ps", bufs=2, space="PSUM") as workp, \
             tc.tile_pool(name="stps", bufs=2, space="PSUM") as stps, \
             tc.tile_pool(name="cps", bufs=2, space="PSUM") as cpsp, \
             tc.tile_pool(name="pt", bufs=4) as ptp, \
             tc.tile_pool(name="ctxn", bufs=6) as ctxp, \
             tc.tile_pool(name="small", bufs=6) as smp, \
             tc.tile_pool(name="outsb", bufs=3) as outp:

            EXP = mybir.ActivationFunctionType.Exp

            def qk_piece(ich, xt, ch):
                i0 = ich * ICH_W
                qps = workp.tile([128, 512], F32, tag="w", space="PSUM")
                for kt in range(KT):
                    nc.tensor.matmul(
                        qps,
                        lhsT=wqk_sb[:, kt, ch * 128:(ch + 1) * 128],
                        rhs=xt[:, kt, :],
                        start=(kt == 0), stop=(kt == KT - 1),
                    )
                nc.vector.tensor_scalar_add(
                    qk_sb[:, ch, i0:i0 + ICH_W], qps, bqk_sb[:, ch:ch + 1])
                if ch == 2:
                    # build ch3=[k3|q3] by partition-swapping ch2=[q3|k3]
                    nc.sync.dma_start(
                        out=qk_sb[64:128, 3, i0:i0 + ICH_W],
                        in_=qk_sb[0:64, 2, i0:i0 + ICH_W])
                    nc.sync.dma_start(
                        out=qk_sb[0:64, 3, i0:i0 + ICH_W],
                        in_=qk_sb[64:128, 2, i0:i0 + ICH_W])

            def proj_pieces(ich):
                """qk^T + v projections for i-chunk ich, one piece per yield.

                Order: ch0, ch1, ch2, v tiles — qk channels first so the
                ch3 swap DMAs are in flight well before the consuming pass
                B; chunk 0 interleaves these into its own pass A (scores
                need ch0/ch1 only, pv_a for block jb needs v tile jb which
                is pumped just before it)."""
                xt = xt_tiles.pop(ich)
                for ch in (0, 1, 2):
                    qk_piece(ich, xt, ch)
                    yield
                for tl in range(ICH_W // 128):
                    tch = ich * (ICH_W // 128) + tl
                    vps = workp.tile([128, 512], F32, tag="w", space="PSUM")
                    for kt in range(KT):
                        nc.tensor.matmul(
                            vps[:, 0:HPC * DK],
                            lhsT=xt[:, kt, tl * 128:(tl + 1) * 128],
                            rhs=wv_sb[:, kt, :],
                            start=(kt == 0), stop=(kt == KT - 1),
                        )
                    nc.vector.tensor_copy(
                        vaug_sb[:, tch, :, 0:DK],
                        vps[:, 0:HPC * DK].rearrange("p (h d) -> p h d",
                                                     h=HPC),
                    )
                    yield

            def outproj_pieces(ich, ctx01, ctx2):
                """Partial out-projection, one 128-token piece per yield."""
                i0 = ich * ICH_W
                for tsub in range(ICH_W // 128):
                    osb = outp.tile([128, D], BF16, tag="osb")
                    for m0, m1 in ((0, 384), (384, D)):
                        ops = workp.tile([128, 512], F32, tag="w",
                                         space="PSUM")
                        nc.tensor.matmul(
                            ops[:, 0:m1 - m0],
                            lhsT=ctx01[:, tsub * 128:(tsub + 1) * 128],
                            rhs=wout01_sb[:, m0:m1],
                            start=True, stop=False)
                        nc.tensor.matmul(
                            ops[:, 0:m1 - m0],
                            lhsT=ctx2[:, tsub * 128:(tsub + 1) * 128],
                            rhs=wout2_sb[:, m0:m1],
                            start=False, stop=True)
                        nc.vector.tensor_copy(osb[:, m0:m1],
                                              ops[:, 0:m1 - m0])
                    nc.sync.dma_start(
                        out=out_d[i0 + tsub * 128:i0 + (tsub + 1) * 128, :],
                        in_=osb)
                    yield

            # head views: (qT, kT) partition slices
            # h0: q=ch0[0:64]   k=ch1[0:64]
            # h1: q=ch0[64:128] k=ch1[64:128]
            # h2 even jb: q=ch2[0:64]  k=ch3[0:64]
            # h2 odd  jb: q=ch3[64:128] k=ch2[64:128]

            def run_gen(gen):
                for _ in gen:
                    pass

            _DONE = object()

            from contextlib import contextmanager

            @contextmanager
            def low_prio():
                """Deprioritize: run only when the PE has nothing else."""
                tc.cur_priority += 1 << 21
                try:
                    yield
                finally:
                    tc.cur_priority -= 1 << 21

            # HAM warmup: the PE sits idle ~7-12us waiting for the first
            # x/weight DMAs, then starts cold (1.2GHz for ~3.4us).  A burst
            # of zero-data matmuls at the lowest priority fills that DMA
            # wait and releases the clock throttle before real work lands.
            # allocated in the LAST SBUF pool: adding tiles to consts would
            # shift every downstream pool (measured ACTIVATE slowdown)
            warm_w = outp.tile([128, 128], BF16, tag="warmw")
            warm_x = outp.tile([128, 512], BF16, tag="warmx")
            nc.vector.memset(warm_w, 0.0)
            nc.vector.memset(warm_x, 0.0)
            with low_prio():
                wps = workp.tile([128, 512], F32, tag="w", space="PSUM")
                for _ in range(12):
                    nc.tensor.matmul(wps, lhsT=warm_w, rhs=warm_x,
                                     start=True, stop=True)

            @contextmanager
            def attn_prio():
                """Priority boost that KEEPS the per-instruction increments
                (tc.high_priority restores cur_priority on exit, so two
                boosted groups separated by one instruction collide at equal
                priority and the scheduler heap scrambles the pairs)."""
                tc.cur_priority -= 1 << 20
                try:
                    yield
                finally:
                    tc.cur_priority += 1 << 20

            def pump(gen):
                """Advance gen one step; return gen or None if exhausted."""
                if gen is not None and next(gen, _DONE) is _DONE:
                    return None
                return gen

            def normalize_den(cbuf, dst):
                """dst[0:64] = cbuf[0:64] / cbuf[64] via approx reciprocal.

                The custom-DVE reciprocal_approx_fast mis-executes when its
                source AP has a nonzero base partition (HW-verified), so the
                denominator row is first copied down to partition 0."""
                den0 = smp.tile([1, ICH_W], F32, tag="den0")
                nc.vector.tensor_copy(den0, cbuf[64:65, :])
                recip = smp.tile([1, ICH_W], F32, tag="recip")
                nc.vector.reciprocal_approx_fast(recip, den0)
                rb = smp.tile([64, ICH_W], F32, tag="rb")
                nc.gpsimd.partition_broadcast(rb, recip)
                nc.vector.tensor_mul(dst, cbuf[0:64, :], rb)

            def scores_a_for(ich2, jb):
                """Pass-A score pair + exp for block jb of chunk ich2."""
                i02 = ich2 * ICH_W
                njb2 = (i02 + ICH_W) // JB_W
                s = jb - (njb2 - 4)
                w0 = 128 * s if s > 0 else 0
                j0 = jb * JB_W
                st = stps.tile([128, 2, ICH_W], F32, tag="st",
                               space="PSUM")
                # high priority keeps the row-packed pair adjacent in the
                # PE queue (a ready out-proj/proj matmul would otherwise
                # split it, serializing the two concurrent halves)
                with attn_prio():
                    nc.tensor.matmul(
                        st[:, 0, w0:],
                        lhsT=qk_sb[0:64, 1, j0:j0 + JB_W],
                        rhs=qk_sb[0:64, 0, i02 + w0:i02 + ICH_W],
                        start=True, stop=True)
                    nc.tensor.matmul(
                        st[:, 1, w0:],
                        lhsT=qk_sb[64:128, 1, j0:j0 + JB_W],
                        rhs=qk_sb[64:128, 0, i02 + w0:i02 + ICH_W],
                        start=True, stop=True)
                pt = ptp.tile([128, 2, ICH_W], BF16, tag="pt")
                nc.scalar.activation(pt[:, :, w0:], st[:, :, w0:], EXP,
                                     bias=0.0, scale=1.0 / np.sqrt(DK))
                return pt

            # chunk 0: emit just ch0/ch1 projections, then interleave the
            # v tiles into pass A (pv for block jb only needs v tile jb,
            # pumped one block earlier) so exp starts ~4us sooner
            gen0 = proj_pieces(0)
            gen0 = pump(gen0)
            gen0 = pump(gen0)
            pend_out = None                      # outproj gen of prev chunk
            hs_pt = None                         # head-start pts for chunk i
            hs_pt2 = None
            for ich in range(n_ich):
                i0 = ich * ICH_W
                emit_xt_dma(ich + 2)
                njb = (i0 + ICH_W) // JB_W     # causal: j-blocks 0..njb-1

                def sw(jb):
                    s = jb - (njb - 4)          # diag position if >= 0
                    return s, (128 * s if s > 0 else 0)

                # ---- pass A: heads 0/1 row-group paired, software-
                # pipelined: scores+exp one block ahead of mask+pv;
                # outproj of the previous chunk interleaved so ScalarE
                # (exp) keeps running while PE does out-proj matmuls ----
                cps0 = cpsp.tile([65, ICH_W], F32, tag="cps", space="PSUM")
                cps1 = cpsp.tile([65, ICH_W], F32, tag="cps", space="PSUM")

                def pv_a(jb, pt):
                    s, w0 = sw(jb)
                    if s >= 0:
                        for hh in range(2):
                            nc.gpsimd.affine_select(
                                out=pt[:, hh, w0:w0 + 128],
                                in_=pt[:, hh, w0:w0 + 128],
                                compare_op=mybir.AluOpType.is_ge,
                                fill=0.0, base=0, pattern=[[1, 128]],
                                channel_multiplier=-1)
                    with attn_prio():
                        nc.tensor.matmul(
                            cps0[:, w0:], lhsT=vaug_sb[:, jb, 0, :],
                            rhs=pt[:, 0, w0:],
                            start=(jb == 0), stop=(jb == njb - 1))
                        nc.tensor.matmul(
                            cps1[:, w0:], lhsT=vaug_sb[:, jb, 1, :],
                            rhs=pt[:, 1, w0:],
                            start=(jb == 0), stop=(jb == njb - 1))

                pend = None
                for jb in range(njb):
                    if jb == 0 and hs_pt is not None:
                        pt = hs_pt          # emitted during prev pass B
                        hs_pt = None
                    elif jb == 1 and hs_pt2 is not None:
                        pt = hs_pt2
                        hs_pt2 = None
                    else:
                        pt = scores_a_for(ich, jb)
                    if ich == 0:
                        # pump BEFORE pv: pv_a(jb-1) needs v tile jb-1,
                        # which is the piece pumped in this iteration
                        gen0 = pump(gen0)
                    if pend is not None:
                        pv_a(pend[0], pend[1])
                    pend = (jb, pt)
                    if ich > 0 and jb in (2, 4):
                        # two out-proj pieces of the previous chunk here,
                        # two more in pass B: spreading them keeps the PE
                        # from locally outrunning ScalarE in pass A
                        pend_out = pump(pend_out)
                while gen0 is not None:           # chunk 0: drain v tiles
                    gen0 = pump(gen0)
                pv_a(pend[0], pend[1])

                # evacuate pass-A accumulators so pass B reuses their PSUM,
                # then normalize h0/h1 right away (DVE is idle during pass B)
                cbuf0 = smp.tile([65, ICH_W], F32, tag="cbuf")
                nc.vector.tensor_copy(cbuf0, cps0)
                cbuf1 = smp.tile([65, ICH_W], F32, tag="cbuf")
                nc.vector.tensor_copy(cbuf1, cps1)
                ctx01 = ctxp.tile([128, ICH_W], BF16, tag="c01")
                cn1 = ctxp.tile([64, ICH_W], BF16, tag="cn1")
                normalize_den(cbuf0, ctx01[0:64, :])
                normalize_den(cbuf1, cn1)
                nc.sync.dma_start(out=ctx01[64:128, :], in_=cn1)

                # ---- pass B: head 2, two j-blocks per group with q/k on
                # alternating partition halves (row-group packing) ----
                cps2 = cpsp.tile([65, ICH_W], F32, tag="cps", space="PSUM")

                def scores_b(grp):
                    st = stps.tile([128, 2, ICH_W], F32, tag="st",
                                   space="PSUM")
                    pt = ptp.tile([128, 2, ICH_W], BF16, tag="pt")
                    # both blocks' matmuls span from the pair's smaller w0 so
                    # one activation covers the tile; the extra columns of
                    # the later diagonal block are finite garbage that pv_b
                    # never reads (it slices per-block from its own w0)
                    wmin = min(sw(grp * 2)[1], sw(grp * 2 + 1)[1])
                    with attn_prio():
                        for jj in range(2):
                            jb = grp * 2 + jj
                            j0 = jb * JB_W
                            if jb % 2 == 0:
                                lhsT = qk_sb[0:64, 3, j0:j0 + JB_W]
                                rhs = qk_sb[0:64, 2, i0 + wmin:i0 + ICH_W]
                            else:
                                lhsT = qk_sb[64:128, 2, j0:j0 + JB_W]
                                rhs = qk_sb[64:128, 3, i0 + wmin:i0 + ICH_W]
                            nc.tensor.matmul(st[:, jj, wmin:], lhsT=lhsT,
                                             rhs=rhs, start=True, stop=True)
                    nc.scalar.activation(
                        pt[:, :, wmin:], st[:, :, wmin:], EXP,
                        bias=0.0, scale=1.0 / np.sqrt(DK))
                    return pt

                def pv_b(grp, pt):
                    for jj in range(2):
                        jb = grp * 2 + jj
                        s, w0 = sw(jb)
                        if s >= 0:
                            nc.gpsimd.affine_select(
                                out=pt[:, jj, w0:w0 + 128],
                                in_=pt[:, jj, w0:w0 + 128],
                                compare_op=mybir.AluOpType.is_ge,
                                fill=0.0, base=0, pattern=[[1, 128]],
                                channel_multiplier=-1)
                        with attn_prio():
                            nc.tensor.matmul(
                                cps2[:, w0:], lhsT=vaug_sb[:, jb, 2, :],
                                rhs=pt[:, jj, w0:],
                                start=(jb == 0), stop=(jb == njb - 1))

                # next chunk's projections interleave into pass B so the
                # PE reaches pass A of chunk i+1 (and its exps) with no
                # projection-only window in between
                proj_gen = (proj_pieces(ich + 1)
                            if ich + 1 < n_ich else None)
                pendB = None
                for grp in range(njb // 2):
                    pt = scores_b(grp)
                    if pendB is not None:
                        pv_b(pendB[0], pendB[1])
                    pendB = (grp, pt)
                    proj_gen = pump(proj_gen)
                    if grp in (1, 3):
                        pend_out = pump(pend_out)
                # head-start: first score block (+exp) of the next chunk's
                # pass A, emitted here so ScalarE has no gap across the
                # pass-B -> pass-A handoff
                if ich + 1 < n_ich:
                    hs_pt = scores_a_for(ich + 1, 0)
                    hs_pt2 = scores_a_for(ich + 1, 1)
                pv_b(pendB[0], pendB[1])
                while pend_out is not None:       # drain leftover out-proj
                    pend_out = pump(pend_out)
                while proj_gen is not None:       # drain if njb was small
                    proj_gen = pump(proj_gen)

                # ---- normalize h2 ----
                cbuf2 = smp.tile([65, ICH_W], F32, tag="cbuf")
                nc.vector.tensor_copy(cbuf2, cps2)
                ctx2 = ctxp.tile([128, ICH_W], BF16, tag="c2")
                nc.vector.memset(ctx2[64:128, :], 0.0)
                normalize_den(cbuf2, ctx2[0:64, :])

                pend_out = outproj_pieces(ich, ctx01, ctx2)

            if pend_out is not None:
                run_gen(pend_out)

    nc.compile()
    return nc


def _to_bf16(a):
    return np.ascontiguousarray(np.asarray(a).astype(NPBF16))


def make_core_inputs(xt_b16, W_qkv, b_qkv, W_out, hg):
    """Host-side weight slicing/permutation for one head-group hg (0..3)."""
    heads = [hg * HPC + i for i in range(HPC)]
    # W_qkv last-dim layout: c = h*192 + s*64 + d  (s: 0=q 1=k 2=v)
    def cols(h, s):
        return slice(h * 192 + s * 64, h * 192 + s * 64 + 64)

    q = [np.asarray(W_qkv[:, cols(h, 0)]) for h in heads]
    k = [np.asarray(W_qkv[:, cols(h, 1)]) for h in heads]
    v = [np.asarray(W_qkv[:, cols(h, 2)]) for h in heads]
    bq = [np.asarray(b_qkv[cols(h, 0)], np.float32) for h in heads]

    zw = np.zeros((W_qkv.shape[0], 128), np.float32)
    wqk = np.concatenate([q[0], q[1], k[0], k[1], q[2], k[2], zw], axis=1)
    z = np.zeros(64, np.float32)
    bqk = np.concatenate([bq[0], bq[1], z, z, bq[2], z, z, z]).astype(
        np.float32)
    wv = np.concatenate(v, axis=1)
    wout = np.concatenate(
        [np.asarray(W_out[h * DK:(h + 1) * DK, :]) for h in heads], axis=0)
    return {
        "xt": xt_b16,
        "wqk": _to_bf16(wqk),
        "bqk": np.ascontiguousarray(bqk),
        "wv": _to_bf16(wv),
        "wout": _to_bf16(wout),
    }


_CACHE = {}


def _get_program(t=T):
    if t not in _CACHE:
        _CACHE[t] = build_program(t)
    return _CACHE[t]


def run_cores(inputs, t=T, trace=False):
    nc = _get_program(t)
    x = np.asarray(inputs["x"], np.float32)
    xt_b16 = [np.ascontiguousarray(x[b].T.astype(NPBF16)) for b in range(B)]
    in_maps = []
    for core in range(N_CORES):
        b, hg = core // 4, core % 4
        in_maps.append(make_core_inputs(xt_b16[b], inputs["W_qkv"],
                                        inputs["b_qkv"], inputs["W_out"], hg))
    res = run_bass_kernel_spmd(nc, in_maps, list(range(N_CORES)), trace=trace)
    return res


def gather(inputs, results):
    b_qkv = np.asarray(inputs["b_qkv"], np.float32)
    W_out = np.asarray(inputs["W_out"], np.float32)
    b_out = np.asarray(inputs["b_out"], np.float32)
    bv = np.concatenate([b_qkv[h * 192 + 128:h * 192 + 192] for h in range(H)])
    fold = bv @ W_out + b_out                      # [D]
    t = results[0]["out"].shape[0]
    out = np.zeros((B, t, D), np.float32)
    for core in range(N_CORES):
        out[core // 4] += np.asarray(results[core]["out"], np.float32)
    out += fold[None, None, :]
    return out


def kernel(**inputs):
    res = run_cores(inputs)
    return gather(inputs, res.results)


if __name__ == "__main__":
    # smoke test with random data
    rng = np.random.default_rng(0)
    inputs = {
        "x": rng.standard_normal((B, T, D), dtype=np.float32),
        "mask": np.triu(np.ones((T, T), dtype=bool), k=1),
        "W_qkv": (rng.standard_normal((D, 3 * D), dtype=np.float32)
                  / np.sqrt(D)),
        "b_qkv": rng.standard_normal(3 * D).astype(np.float32) * 0.02,
        "W_out": (rng.standard_normal((D, D), dtype=np.float32)
                  / np.sqrt(D)),
        "b_out": rng.standard_normal(D).astype(np.float32) * 0.02,
    }
    out = kernel(**inputs)
    print(out.shape, out.dtype)


# revision 55
# speedup vs baseline: 1.1831x; 1.1831x over previous
"""Trainium2 Bass kernel for causal multi-head attention.

Problem: B=2, T=4096, D=768, H=12 heads, d_k=64, causal mask.
Sharding: 8 cores = 2 batches x 4 head-groups (3 heads each).

v8 design (all-bf16 on device, ~305us vs 400us v2 baseline):
- Host ships x^T (pre-transposed, bf16); weights pre-sliced per head-group.
- One fused loop per 512-query i-chunk: project qk^T/v, flash-style causal
  attention with transposed scores (S^T = k q^T), partial out-projection.
- qk projection ships only 3 channels (ch0=[q1|q2] ch1=[k1|k2] ch2=[q3|k3]);
  the partition-swapped ch3=[k3|q3] needed for pass-B row-packing is built
  by two SBUF partition-shift DMAs instead of 6 extra matmuls per chunk.
- Normalization uses reciprocal_approx_fast (1 DVE op, ~51 ULP) instead of
  the 6-cyc/elem iterative reciprocal that blocked the DVE queue; its src
  is first copied to partition 0 (custom-DVE ops mis-execute on nonzero
  base partitions, HW-verified).
- Deep cross-chunk software pipeline: out-proj of chunk i pumps into pass A
  of chunk i+1, projections of chunk i+1 pump into pass B of chunk i, and
  pass A of chunk i+1 gets a head-start score block inside pass B of chunk
  i, so ScalarE (exp, the ~219us bottleneck engine) stays fed.
- Attention matmuls carry a scheduler priority boost that keeps row-packed
  score pairs adjacent (the Tile priority heap otherwise splits ~half of
  them with ready out-proj matmuls, serializing the concurrent halves).
- Causal handling at 128-block granularity; the true-diagonal block is
  masked by gpsimd affine_select after exp.
- Host sums the 4 head-group partials per batch and adds the folded bias
  constant (v-bias @ W_out + b_out).  k-bias dropped (softmax-invariant).

Self-contained: hardcodes all shapes; only imports the concourse runtime.
"""

import sys

sys.path.insert(0, "/opt/trn_rl_repo")

from contextlib import ExitStack

import numpy as np
import ml_dtypes

import concourse.bass as bass
import concourse.mybir as mybir
import concourse.tile as tile
from concourse import bacc
from concourse.bass_utils import run_bass_kernel_spmd

F32 = mybir.dt.float32
BF16 = mybir.dt.bfloat16
NPBF16 = ml_dtypes.bfloat16

B, T, D = 2, 4096, 768
H, DK = 12, 64
HPC = 3          # heads per core
N_CORES = 8
ICH_W = 512      # i-chunk width (queries per outer step)
JB_W = 128       # j-block width (keys per matmul)
KT = D // 128    # 6 contraction tiles for the projections
USE_RS = False


def build_program(t=T, use_rs=USE_RS):
    """Build the SPMD Bass program for one core (all cores identical)."""
    n_ich = t // ICH_W
    n_tch = t // 128

    nc = bacc.Bacc("TRN2", target_bir_lowering=False, debug=False,
                   num_devices=N_CORES)

    # x^T: [D, t] bf16, row-major (row stride t)
    xt_d = nc.dram_tensor("xt", [D, t], BF16, kind="ExternalInput").ap()
    # qk projection weights, 3 chunks of 128 output channels:
    # ch0=[q1|q2] ch1=[k1|k2] ch2=[q3|k3]; host-padded to 512 columns so
    # the SBUF tile keeps the 4-channel footprint (shrinking it shifts all
    # downstream pools and was measured to slow ACTIVATE 1.0->1.22 cyc/elem)
    # and the DMA stays fully contiguous
    wqk_d = nc.dram_tensor("wqk", [D, 512], BF16, kind="ExternalInput").ap()
    bqk_d = nc.dram_tensor("bqk", [512], F32, kind="ExternalInput").ap()
    wv_d = nc.dram_tensor("wv", [D, HPC * DK], BF16, kind="ExternalInput").ap()
    wout_d = nc.dram_tensor("wout", [HPC * DK, D], BF16,
                            kind="ExternalInput").ap()
    out_d = nc.dram_tensor("out", [t, D], BF16, kind="ExternalOutput").ap()

    with tile.TileContext(nc) as tc, ExitStack() as top:
        consts = top.enter_context(tc.tile_pool(name="consts", bufs=1))
        persist = top.enter_context(tc.tile_pool(name="persist", bufs=1))

        # q^T / k^T per chunk: [128, 4, t] bf16 (ch3 filled by DMA shifts)
        qk_sb = persist.tile([128, 4, t], BF16)
        # v (natural layout) + ones column: [128, n_tch, HPC, 65] bf16
        vaug_sb = persist.tile([128, n_tch, HPC, DK + 1], BF16)

        xtp = top.enter_context(tc.tile_pool(name="xtp", bufs=3))
        xt_tiles = {}

        def emit_xt_dma(ich):
            if ich >= n_ich:
                return
            i0 = ich * ICH_W
            xt = xtp.tile([128, KT, ICH_W], BF16, tag="xt")
            nc.sync.dma_start(
                out=xt,
                in_=xt_d[:, i0:i0 + ICH_W].rearrange(
                    "(kt p) i -> p kt i", p=128))
            xt_tiles[ich] = xt

        # DMA issue order favors the critical path to the first exp:
        # wqk + bqk + xt0 (first qk projections), then everything else
        wqk_sb = consts.tile([128, KT, 512], BF16)
        nc.sync.dma_start(out=wqk_sb,
                          in_=wqk_d.rearrange("(kt p) c -> p kt c", p=128))
        bqk_sb = consts.tile([128, 4], F32)
        nc.sync.dma_start(out=bqk_sb, in_=bqk_d.rearrange("(ch p) -> p ch",
                                                          p=128))
        emit_xt_dma(0)
        wv_sb = consts.tile([128, KT, HPC * DK], BF16)
        nc.sync.dma_start(out=wv_sb,
                          in_=wv_d.rearrange("(kt p) c -> p kt c", p=128))
        emit_xt_dma(1)
        # h0|h1 stacked on 128 partitions (one K=128 out-proj matmul); h2's
        # weights zero-padded to 128 rows so its matmul also runs K=128
        wout01_sb = consts.tile([128, D], BF16)
        nc.sync.dma_start(out=wout01_sb, in_=wout_d[0:128, :])
        wout2_sb = consts.tile([128, D], BF16)
        nc.vector.memset(wout2_sb[64:128, :], 0.0)
        nc.sync.dma_start(out=wout2_sb[0:64, :], in_=wout_d[128:192, :])

        # ones column for the v-augmentation (denominator row), set once
        nc.vector.memset(vaug_sb[:, :, :, DK:DK + 1], 1.0)

        with tc.tile_pool(name="work_ps", bufs=2, space="PSUM") as workp, \
             tc.tile_pool(name="stps", bufs=2, space="PSUM") as stps, \
             tc.tile_pool(name="cps", bufs=2, space="PSUM") as cpsp, \
             tc.tile_pool(name="pt", bufs=3) as ptp, \
             tc.tile_pool(name="ctxn", bufs=6) as ctxp, \
             tc.tile_pool(name="small", bufs=6) as smp, \
             tc.tile_pool(name="outsb", bufs=3) as outp:

            EXP = mybir.ActivationFunctionType.Exp

            def qk_piece(ich, xt, ch):
                i0 = ich * ICH_W
                qps = workp.tile([128, 512], F32, tag="w", space="PSUM")
                for kt in range(KT):
                    nc.tensor.matmul(
                        qps,
                        lhsT=wqk_sb[:, kt, ch * 128:(ch + 1) * 128],
                        rhs=xt[:, kt, :],
                        start=(kt == 0), stop=(kt == KT - 1),
                    )
                nc.vector.tensor_scalar_add(
                    qk_sb[:, ch, i0:i0 + ICH_W], qps, bqk_sb[:, ch:ch + 1])
                if ch == 2:
                    # build ch3=[k3|q3] by partition-swapping ch2=[q3|k3]
                    nc.sync.dma_start(
                        out=qk_sb[64:128, 3, i0:i0 + ICH_W],
                        in_=qk_sb[0:64, 2, i0:i0 + ICH_W])
                    nc.sync.dma_start(
                        out=qk_sb[0:64, 3, i0:i0 + ICH_W],
                        in_=qk_sb[64:128, 2, i0:i0 + ICH_W])

            def proj_pieces(ich):
                """qk^T + v projections for i-chunk ich, one piece per yield.

                Order: ch0, ch1, ch2, v tiles — qk channels first so the
                ch3 swap DMAs are in flight well before the consuming pass
                B; chunk 0 interleaves these into its own pass A (scores
                need ch0/ch1 only, pv_a for block jb needs v tile jb which
                is pumped just before it)."""
                xt = xt_tiles.pop(ich)
                for ch in (0, 1, 2):
                    qk_piece(ich, xt, ch)
                    yield
                for tl in range(ICH_W // 128):
                    tch = ich * (ICH_W // 128) + tl
                    vps = workp.tile([128, 512], F32, tag="w", space="PSUM")
                    for kt in range(KT):
                        nc.tensor.matmul(
                            vps[:, 0:HPC * DK],
                            lhsT=xt[:, kt, tl * 128:(tl + 1) * 128],
                            rhs=wv_sb[:, kt, :],
                            start=(kt == 0), stop=(kt == KT - 1),
                        )
                    nc.vector.tensor_copy(
                        vaug_sb[:, tch, :, 0:DK],
                        vps[:, 0:HPC * DK].rearrange("p (h d) -> p h d",
                                                     h=HPC),
                    )
                    yield

            def outproj_pieces(ich, ctx01, ctx2):
                """Partial out-projection, one 128-token piece per yield."""
                i0 = ich * ICH_W
                for tsub in range(ICH_W // 128):
                    osb = outp.tile([128, D], BF16, tag="osb")
                    for m0, m1 in ((0, 384), (384, D)):
                        ops = workp.tile([128, 512], F32, tag="w",
                                         space="PSUM")
                        nc.tensor.matmul(
                            ops[:, 0:m1 - m0],
                            lhsT=ctx01[:, tsub * 128:(tsub + 1) * 128],
                            rhs=wout01_sb[:, m0:m1],
                            start=True, stop=False)
                        nc.tensor.matmul(
                            ops[:, 0:m1 - m0],
                            lhsT=ctx2[:, tsub * 128:(tsub + 1) * 128],
                            rhs=wout2_sb[:, m0:m1],
                            start=False, stop=True)
                        nc.vector.tensor_copy(osb[:, m0:m1],
                                              ops[:, 0:m1 - m0])
                    nc.sync.dma_start(
                        out=out_d[i0 + tsub * 128:i0 + (tsub + 1) * 128, :],
                        in_=osb)
                    yield

            # head views: (qT, kT) partition slices
            # h0: q=ch0[0:64]   k=ch1[0:64]
            # h1: q=ch0[64:128] k=ch1[64:128]
            # h2 even jb: q=ch2[0:64]  k=ch3[0:64]
            # h2 odd  jb: q=ch3[64:128] k=ch2[64:128]

            def run_gen(gen):
                for _ in gen:
                    pass

            _DONE = object()

            from contextlib import contextmanager

            @contextmanager
            def low_prio():
                """Deprioritize: run only when the PE has nothing else."""
                tc.cur_priority += 1 << 21
                try:
                    yield
                finally:
                    tc.cur_priority -= 1 << 21

            # HAM warmup: the PE sits idle ~7-12us waiting for the first
            # x/weight DMAs, then starts cold (1.2GHz for ~3.4us).  A burst
            # of zero-data matmuls at the lowest priority fills that DMA
            # wait and releases the clock throttle before real work lands.
            # allocated in the LAST SBUF pool: adding tiles to consts would
            # shift every downstream pool (measured ACTIVATE slowdown)
            warm_w = outp.tile([128, 128], BF16, tag="warmw")
            warm_x = outp.tile([128, 512], BF16, tag="warmx")
            nc.vector.memset(warm_w, 0.0)
            nc.vector.memset(warm_x, 0.0)
            with low_prio():
                wps = workp.tile([128, 512], F32, tag="w", space="PSUM")
                for _ in range(12):
                    nc.tensor.matmul(wps, lhsT=warm_w, rhs=warm_x,
                                     start=True, stop=True)

            @contextmanager
            def attn_prio():
                """Priority boost that KEEPS the per-instruction increments
                (tc.high_priority restores cur_priority on exit, so two
                boosted groups separated by one instruction collide at equal
                priority and the scheduler heap scrambles the pairs)."""
                tc.cur_priority -= 1 << 20
                try:
                    yield
                finally:
                    tc.cur_priority += 1 << 20

            def pump(gen):
                """Advance gen one step; return gen or None if exhausted."""
                if gen is not None and next(gen, _DONE) is _DONE:
                    return None
                return gen

            def normalize_den(cbuf, dst):
                """dst[0:64] = cbuf[0:64] / cbuf[64] via approx reciprocal.

                The custom-DVE reciprocal_approx_fast mis-executes when its
                source AP has a nonzero base partition (HW-verified), so the
                denominator row is first copied down to partition 0."""
                den0 = smp.tile([1, ICH_W], F32, tag="den0")
                nc.vector.tensor_copy(den0, cbuf[64:65, :])
                recip = smp.tile([1, ICH_W], F32, tag="recip")
                nc.vector.reciprocal_approx_fast(recip, den0)
                rb = smp.tile([64, ICH_W], F32, tag="rb")
                nc.gpsimd.partition_broadcast(rb, recip)
                nc.vector.tensor_mul(dst, cbuf[0:64, :], rb)

            def scores_a_for(ich2, jb):
                """Pass-A score pair + exp for block jb of chunk ich2."""
                i02 = ich2 * ICH_W
                njb2 = (i02 + ICH_W) // JB_W
                s = jb - (njb2 - 4)
                w0 = 128 * s if s > 0 else 0
                j0 = jb * JB_W
                st = stps.tile([128, 2, ICH_W], F32, tag="st",
                               space="PSUM")
                # high priority keeps the row-packed pair adjacent in the
                # PE queue (a ready out-proj/proj matmul would otherwise
                # split it, serializing the two concurrent halves)
                with attn_prio():
                    nc.tensor.matmul(
                        st[:, 0, w0:],
                        lhsT=qk_sb[0:64, 1, j0:j0 + JB_W],
                        rhs=qk_sb[0:64, 0, i02 + w0:i02 + ICH_W],
                        start=True, stop=True)
                    nc.tensor.matmul(
                        st[:, 1, w0:],
                        lhsT=qk_sb[64:128, 1, j0:j0 + JB_W],
                        rhs=qk_sb[64:128, 0, i02 + w0:i02 + ICH_W],
                        start=True, stop=True)
                pt = ptp.tile([128, 2, ICH_W], BF16, tag="pt")
                nc.scalar.activation(pt[:, :, w0:], st[:, :, w0:], EXP,
                                     bias=0.0, scale=1.0 / np.sqrt(DK))
                return pt

            # chunk 0: emit just ch0/ch1 projections, then interleave the
            # v tiles into pass A (pv for block jb only needs v tile jb,
            # pumped one block earlier) so exp starts ~4us sooner
            gen0 = proj_pieces(0)
            gen0 = pump(gen0)
            gen0 = pump(gen0)
            pend_out = None                      # outproj gen of prev chunk
            hs_pt = None                         # head-start pt for chunk i
            for ich in range(n_ich):
                i0 = ich * ICH_W
                emit_xt_dma(ich + 2)
                njb = (i0 + ICH_W) // JB_W     # causal: j-blocks 0..njb-1

                def sw(jb):
                    s = jb - (njb - 4)          # diag position if >= 0
                    return s, (128 * s if s > 0 else 0)

                # ---- pass A: heads 0/1 row-group paired, software-
                # pipelined: scores+exp one block ahead of mask+pv;
                # outproj of the previous chunk interleaved so ScalarE
                # (exp) keeps running while PE does out-proj matmuls ----
                cps0 = cpsp.tile([65, ICH_W], F32, tag="cps", space="PSUM")
                cps1 = cpsp.tile([65, ICH_W], F32, tag="cps", space="PSUM")

                def pv_a(jb, pt):
                    s, w0 = sw(jb)
                    if s >= 0:
                        for hh in range(2):
                            nc.gpsimd.affine_select(
                                out=pt[:, hh, w0:w0 + 128],
                                in_=pt[:, hh, w0:w0 + 128],
                                compare_op=mybir.AluOpType.is_ge,
                                fill=0.0, base=0, pattern=[[1, 128]],
                                channel_multiplier=-1)
                    with attn_prio():
                        nc.tensor.matmul(
                            cps0[:, w0:], lhsT=vaug_sb[:, jb, 0, :],
                            rhs=pt[:, 0, w0:],
                            start=(jb == 0), stop=(jb == njb - 1))
                        nc.tensor.matmul(
                            cps1[:, w0:], lhsT=vaug_sb[:, jb, 1, :],
                            rhs=pt[:, 1, w0:],
                            start=(jb == 0), stop=(jb == njb - 1))

                pend = None
                for jb in range(njb):
                    if jb == 0 and hs_pt is not None:
                        pt = hs_pt          # emitted during prev pass B
                        hs_pt = None
                    else:
                        pt = scores_a_for(ich, jb)
                    if ich == 0:
                        # pump BEFORE pv: pv_a(jb-1) needs v tile jb-1,
                        # which is the piece pumped in this iteration
                        gen0 = pump(gen0)
                    if pend is not None:
                        pv_a(pend[0], pend[1])
                    pend = (jb, pt)
                    if ich > 0 and jb in (2, 4):
                        # two out-proj pieces of the previous chunk here,
                        # two more in pass B: spreading them keeps the PE
                        # from locally outrunning ScalarE in pass A
                        pend_out = pump(pend_out)
                while gen0 is not None:           # chunk 0: drain v tiles
                    gen0 = pump(gen0)
                pv_a(pend[0], pend[1])

                # evacuate pass-A accumulators so pass B reuses their PSUM,
                # then normalize h0/h1 right away (DVE is idle during pass B)
                cbuf0 = smp.tile([65, ICH_W], F32, tag="cbuf")
                nc.vector.tensor_copy(cbuf0, cps0)
                cbuf1 = smp.tile([65, ICH_W], F32, tag="cbuf")
                nc.vector.tensor_copy(cbuf1, cps1)
                ctx01 = ctxp.tile([128, ICH_W], BF16, tag="c01")
                cn1 = ctxp.tile([64, ICH_W], BF16, tag="cn1")
                normalize_den(cbuf0, ctx01[0:64, :])
                normalize_den(cbuf1, cn1)
                nc.sync.dma_start(out=ctx01[64:128, :], in_=cn1)

                # ---- pass B: head 2, two j-blocks per group with q/k on
                # alternating partition halves (row-group packing) ----
                cps2 = cpsp.tile([65, ICH_W], F32, tag="cps", space="PSUM")

                def scores_b(grp):
                    st = stps.tile([128, 2, ICH_W], F32, tag="st",
                                   space="PSUM")
                    pt = ptp.tile([128, 2, ICH_W], BF16, tag="pt")
                    # both blocks' matmuls span from the pair's smaller w0 so
                    # one activation covers the tile; the extra columns of
                    # the later diagonal block are finite garbage that pv_b
                    # never reads (it slices per-block from its own w0)
                    wmin = min(sw(grp * 2)[1], sw(grp * 2 + 1)[1])
                    with attn_prio():
                        for jj in range(2):
                            jb = grp * 2 + jj
                            j0 = jb * JB_W
                            if jb % 2 == 0:
                                lhsT = qk_sb[0:64, 3, j0:j0 + JB_W]
                                rhs = qk_sb[0:64, 2, i0 + wmin:i0 + ICH_W]
                            else:
                                lhsT = qk_sb[64:128, 2, j0:j0 + JB_W]
                                rhs = qk_sb[64:128, 3, i0 + wmin:i0 + ICH_W]
                            nc.tensor.matmul(st[:, jj, wmin:], lhsT=lhsT,
                                             rhs=rhs, start=True, stop=True)
                    nc.scalar.activation(
                        pt[:, :, wmin:], st[:, :, wmin:], EXP,
                        bias=0.0, scale=1.0 / np.sqrt(DK))
                    return pt

                def pv_b(grp, pt):
                    for jj in range(2):
                        jb = grp * 2 + jj
                        s, w0 = sw(jb)
                        if s >= 0:
                            nc.gpsimd.affine_select(
                                out=pt[:, jj, w0:w0 + 128],
                                in_=pt[:, jj, w0:w0 + 128],
                                compare_op=mybir.AluOpType.is_ge,
                                fill=0.0, base=0, pattern=[[1, 128]],
                                channel_multiplier=-1)
                        with attn_prio():
                            nc.tensor.matmul(
                                cps2[:, w0:], lhsT=vaug_sb[:, jb, 2, :],
                                rhs=pt[:, jj, w0:],
                                start=(jb == 0), stop=(jb == njb - 1))

                # next chunk's projections interleave into pass B so the
                # PE reaches pass A of chunk i+1 (and its exps) with no
                # projection-only window in between
                proj_gen = (proj_pieces(ich + 1)
                            if ich + 1 < n_ich else None)
                pendB = None
                for grp in range(njb // 2):
                    pt = scores_b(grp)
                    if pendB is not None:
                        pv_b(pendB[0], pendB[1])
                    pendB = (grp, pt)
                    proj_gen = pump(proj_gen)
                    if grp in (1, 3):
                        pend_out = pump(pend_out)
                # head-start: first score block (+exp) of the next chunk's
                # pass A, emitted here so ScalarE has no gap across the
                # pass-B -> pass-A handoff
                if ich + 1 < n_ich:
                    hs_pt = scores_a_for(ich + 1, 0)
                pv_b(pendB[0], pendB[1])
                while pend_out is not None:       # drain leftover out-proj
                    pend_out = pump(pend_out)
                while proj_gen is not None:       # drain if njb was small
                    proj_gen = pump(proj_gen)

                # ---- normalize h2 ----
                cbuf2 = smp.tile([65, ICH_W], F32, tag="cbuf")
                nc.vector.tensor_copy(cbuf2, cps2)
                ctx2 = ctxp.tile([128, ICH_W], BF16, tag="c2")
                nc.vector.memset(ctx2[64:128, :], 0.0)
                normalize_den(cbuf2, ctx2[0:64, :])

                pend_out = outproj_pieces(ich, ctx01, ctx2)

            if pend_out is not None:
                run_gen(pend_out)

    nc.compile()
    return nc


def _to_bf16(a):
    return np.ascontiguousarray(np.asarray(a).astype(NPBF16))


def make_core_inputs(xt_b16, W_qkv, b_qkv, W_out, hg):
    """Host-side weight slicing/permutation for one head-group hg (0..3)."""
    heads = [hg * HPC + i for i in range(HPC)]
    # W_qkv last-dim layout: c = h*192 + s*64 + d  (s: 0=q 1=k 2=v)
    def cols(h, s):
        return slice(h * 192 + s * 64, h * 192 + s * 64 + 64)

    q = [np.asarray(W_qkv[:, cols(h, 0)]) for h in heads]
    k = [np.asarray(W_qkv[:, cols(h, 1)]) for h in heads]
    v = [np.asarray(W_qkv[:, cols(h, 2)]) for h in heads]
    bq = [np.asarray(b_qkv[cols(h, 0)], np.float32) for h in heads]

    zw = np.zeros((W_qkv.shape[0], 128), np.float32)
    wqk = np.concatenate([q[0], q[1], k[0], k[1], q[2], k[2], zw], axis=1)
    z = np.zeros(64, np.float32)
    bqk = np.concatenate([bq[0], bq[1], z, z, bq[2], z, z, z]).astype(
        np.float32)
    wv = np.concatenate(v, axis=1)
    wout = np.concatenate(
        [np.asarray(W_out[h * DK:(h + 1) * DK, :]) for h in heads], axis=0)
    return {
        "xt": xt_b16,
        "wqk": _to_bf16(wqk),
        "bqk": np.ascontiguousarray(bqk),
        "wv": _to_bf16(wv),
        "wout": _to_bf16(wout),
    }


_CACHE = {}


def _get_program(t=T):
    if t not in _CACHE:
        _CACHE[t] = build_program(t)
    return _CACHE[t]


def run_cores(inputs, t=T, trace=False):
    nc = _get_program(t)
    x = np.asarray(inputs["x"], np.float32)
    xt_b16 = [np.ascontiguousarray(x[b].T.astype(NPBF16)) for b in range(B)]
    in_maps = []
    for core in range(N_CORES):
        b, hg = core // 4, core % 4
        in_maps.append(make_core_inputs(xt_b16[b], inputs["W_qkv"],
                                        inputs["b_qkv"], inputs["W_out"], hg))
    res = run_bass_kernel_spmd(nc, in_maps, list(range(N_CORES)), trace=trace)
    return res


def gather(inputs, results):
    b_qkv = np.asarray(inputs["b_qkv"], np.float32)
    W_out = np.asarray(inputs["W_out"], np.float32)
    b_out = np.asarray(inputs["b_out"], np.float32)
    bv = np.concatenate([b_qkv[h * 192 + 128:h * 192 + 192] for h in range(H)])
    fold = bv @ W_out + b_out                      # [D]
    t = results[0]["out"].shape[0]
    out = np.zeros((B, t, D), np.float32)
    for core in range(N_CORES):
        out[core // 4] += np.asarray(results[core]["out"], np.float32)
    out += fold[None, None, :]
    return out


def kernel(**inputs):
    res = run_cores(inputs)
    return gather(inputs, res.results)


if __name__ == "__main__":
    # smoke test with random data
    rng = np.random.default_rng(0)
    inputs = {
        "x": rng.standard_normal((B, T, D), dtype=np.float32),
        "mask": np.triu(np.ones((T, T), dtype=bool), k=1),
        "W_qkv": (rng.standard_normal((D, 3 * D), dtype=np.float32)
                  / np.sqrt(D)),
        "b_qkv": rng.standard_normal(3 * D).astype(np.float32) * 0.02,
        "W_out": (rng.standard_normal((D, D), dtype=np.float32)
                  / np.sqrt(D)),
        "b_out": rng.standard_normal(D).astype(np.float32) * 0.02,
    }
    out = kernel(**inputs)
    print(out.shape, out.dtype)
